# revision 3
# baseline (speedup 1.0000x reference)
"""MoE top-k routing + grouped down-proj GEMM + reduce-scatter for trn2 (8 cores).

Problem: intermediate_states [4, 2048, 1024] f16 (rank-sharded expanded-token
activations), w [4, 8, 1024, 2048] f16 (rank-sharded per-expert down-proj),
router_logits [1024, 8] f32, topk=2.  Output [4, 256, 2048] f16.

Strategy: per expanded token tk routed to expert e(tk),
y[tk] = (gate(tk)*x_full[tk]) @ w_full[e(tk)], with x_full [TK, 4096] (rank dim
folded into the contraction) and w_full[e] [4096, 2048].  The fp32 gate is
folded into x on the host (f16 rounding costs ~5e-4 rel), so the device is a
pure grouped GEMM.

Work = jobs of (one 128-token tile of one expert) x (one K-half of 2048),
token-stationary: per (job, ks) one LDWEIGHTS (x tile [128K, 128tok]) feeds 4
N=512 matmuls streaming the W slice — the 1:4 LDW:MM ratio keeps the PE at
its 213ns/512-col roofline (1 LDW per MM would pay ~28ns/pair of stationary
swap overhead).  Jobs 0-2 share W slice A, jobs 3-4 share slice B (a slice =
one (expert, K-half) [2048, 2048] f16 block); host packs the (expert, K-half)
groups into the 8 cores' A/B slots (sum(ceil(c_e/128)) <= 23, c_e <= 384).

Schedule: slice A's three jobs run interleaved in two H-half passes (3 token
tiles x 2 PSUM banks live; W demand ~205 GB/s, far enough under the ~358 GB/s
per-core DMA rate that the PE never starves — a 2-job full-H interleave
demands ~360 GB/s and stalls), with pass 2 re-reading the resident slice.
Job 3 then runs in two H-half passes and job 4 full-H on resident slice B, so
every eviction except job 4's overlaps later matmuls; job 4 evicts in
quarters with output DMAs split across the sync and scalar HWDGE queues to
pipeline the last bytes out.  All input DMAs form one self-pacing stream on
the sync queue in exact consumption order.  Warm-up matmuls on zeros cover
the PE HAM ramp (cold 1.2 GHz until ~3.4us of continuous busy) while the
first W/x pieces land.  Host sums each token's 4 partial rows (topk=2 experts
x 2 K-halves).

Fallback for pathological routing (an expert with >384 tokens): expert-per-
core kernel with full K=4096 and capacity padded to 128, launched as many
times as needed.
"""

import numpy as np

R, T_TOK, TOPK, E = 4, 1024, 2, 8
I_PR, H = 1024, 2048
K = R * I_PR            # 4096 contraction
P = 128
NF = 512                # matmul free-dim (one PSUM bank of fp32)
NH = H // NF            # 4
N_CORES = 8

# job mode
KH = K // 2             # 2048 per K-half
KS2 = KH // P           # 16 k-subtiles per job
JOBS = 5                # jobs per core: 0-2 -> W slice A, 3-4 -> W slice B
SLOT_OF_JOB = (0, 0, 0, 1, 1)
SLOT_CAP = (3, 2)
NWARM = 11

# fallback (expert-per-core) mode
KSUB = K // P           # 32
CAP_FB = 384            # token capacity per launch in fallback mode

_prog_cache: dict[str, object] = {}


def _new_bacc():
    from concourse import bacc

    return bacc.Bacc(
        "TRN2",
        target_bir_lowering=False,
        debug=False,
        num_devices=N_CORES,
    )


def _build_program_jobs():
    import concourse.mybir as mybir
    import concourse.tile as tile

    f16 = mybir.dt.float16
    f32 = mybir.dt.float32

    nc = _new_bacc()
    # xj[j, p, ks*P + m] = gated x of job-j token m at K-row ks*P + p of the
    # job's K-half: the SBUF stationary layout, so loading is a plain 2D DMA.
    xj = nc.declare_dram_parameter("xj", [JOBS, P, KS2 * P], f16, isOutput=False)
    wh = nc.declare_dram_parameter("wh", [2, KS2, P, H], f16, isOutput=False)
    ho = nc.declare_dram_parameter("ho", [JOBS, P, H], f16, isOutput=True)

    with tile.TileContext(nc) as tc:
        with tc.tile_pool(name="sb", bufs=1) as sb, \
             tc.tile_pool(name="ps", bufs=2, space="PSUM") as psp:
            # Two resident W slices (32 KB/partition each); DMAed in per-ks
            # 512 KB pieces in consumption order on the sync queue.  x tiles
            # ride the scalar HWDGE queue (separate physical ring) in halves.
            wt = [sb.tile([P, KS2 * H], f16, name=f"w{s}", tag=f"w{s}", bufs=1)
                  for s in range(2)]
            xt = [sb.tile([P, KS2 * P], f16, name=f"x{j}", tag=f"x{j}", bufs=1)
                  for j in range(JOBS)]

            HXB = KS2 * P // 2  # half of an x tile's free dim

            def dma_x(j, half, eng=None):
                sl = slice(half * HXB, (half + 1) * HXB)
                (eng or nc.sync).dma_start(xt[j][:, sl], xj[j, :, sl])

            def dma_w(s, ks):
                nc.sync.dma_start(wt[s][:, ks * H:(ks + 1) * H], wh[s, ks, :, :])

            def dma_w_piece(s, ks, half, eng=None):
                sl = slice(ks * H + half * (H // 2), ks * H + (half + 1) * (H // 2))
                (eng or nc.sync).dma_start(wt[s][:, sl],
                                           wh[s, ks, :, half * (H // 2):
                                              (half + 1) * (H // 2)])

            # One self-pacing stream on the sync queue, in consumption order.
            # Slice A is DMAed as 32 H-half pieces (256 KB each) matching the
            # two H-half passes of the A-era compute, which only consumes W
            # at ~205 GB/s — comfortably under the ~358 GB/s per-core DMA
            # rate, so the PE never starves (the V1-V4 traces showed the
            # 2-job full-H interleave demanding ~360 GB/s and stalling).
            # x pieces sit just ahead of their consumers.
            # All input pieces stay on the sync queue: the scalar HWDGE ring
            # starts late (its preamble includes a ~1us ACT_TABLE_LOAD) and
            # is jittery at the head — V8/V9 runs that put early W pieces
            # there produced 2-3us stalls on one core per run.
            dma_w_piece(0, 0, 0)
            dma_x(0, 0)
            dma_x(1, 0)
            dma_w_piece(0, 1, 0)
            dma_x(2, 0)
            for ks in range(2, 8):
                dma_w_piece(0, ks, 0)
            # x second halves feed ks8+ of the pass (~8us later) — keep them
            # behind the W pieces the PE needs first.
            dma_x(0, 1)
            dma_x(1, 1)
            dma_x(2, 1)
            for ks in range(8, KS2):
                dma_w_piece(0, ks, 0)
            for ks in range(KS2):
                dma_w_piece(0, ks, 1)
            dma_w(1, 0)
            dma_w(1, 1)
            dma_x(3, 0)
            dma_w(1, 2)
            dma_w(1, 3)
            dma_x(3, 1)
            for ks in range(4, 8):
                dma_w(1, ks)
            dma_x(4, 0)
            dma_w(1, 8)
            dma_x(4, 1)
            for ks in range(9, KS2):
                dma_w(1, ks)

            HHF = H // 2
            ot = [sb.tile([P, H], f16, name=f"o{j}", tag=f"o{j}", bufs=1)
                  for j in range(JOBS)]
            ev_cnt = [0]

            def cast_out(dst, src):
                # pure f32->f16 cast (gate already folded into x); alternate
                # scalar/vector so adjacent evictions run in parallel.
                if ev_cnt[0] % 2:
                    nc.vector.tensor_scalar_mul(dst, src, 1.0)
                else:
                    nc.scalar.copy(dst, src)
                ev_cnt[0] += 1

            # No warm-up matmuls: the first ~3.4us of real matmuls run at the
            # HAM-cold 1.2 GHz (427 ns each instead of 215) which costs ~1.7us
            # of effective time, but dropping the 11-matmul zero warm-up block
            # (~3.9us serial, gated on a memset) nets ~2us and lets the input
            # DMA stream build a lead while the cold matmuls self-pace.

            # Slice A (jobs 0-2) in two H-half passes: 3 token tiles
            # interleaved per (ks, H-half) keep only 3x2 PSUM banks live and
            # consume W at ~205 GB/s; pass 2 re-reads the resident slice.
            for hh in range(2):
                ps_a = [psp.tile([P, HHF], f32, name=f"psA{hh}_{t}", tag="ps",
                                 bufs=4) for t in range(3)]
                for ks in range(KS2):
                    for t in range(3):
                        lhs = xt[t][:, ks * P:(ks + 1) * P]
                        for h2 in range(2):
                            nc.tensor.matmul(
                                ps_a[t][:, h2 * NF:(h2 + 1) * NF],
                                lhsT=lhs,
                                rhs=wt[0][:, ks * H + hh * HHF + h2 * NF:
                                          ks * H + hh * HHF + (h2 + 1) * NF],
                                start=(ks == 0),
                                stop=(ks == KS2 - 1),
                            )
                for t in range(3):
                    dst = ot[t][:, hh * HHF:(hh + 1) * HHF]
                    cast_out(dst, ps_a[t][:])
                    nc.sync.dma_start(ho[t, :, hh * HHF:(hh + 1) * HHF], dst)

            # Slice B: job 3 in two H-half passes (each PSUM allocation's
            # predecessor buffer is then long-evicted — no pool stalls), its
            # evictions+DMAs overlapping job 4, which runs full-H and evicts
            # in quarters with output DMAs split across the sync and scalar
            # HWDGE queues to pipeline the last bytes out.
            for hh in range(2):
                ps3 = psp.tile([P, HHF], f32, name=f"ps3_{hh}", tag="ps",
                               bufs=4)
                for ks in range(KS2):
                    lhs = xt[3][:, ks * P:(ks + 1) * P]
                    for h2 in range(2):
                        nc.tensor.matmul(
                            ps3[:, h2 * NF:(h2 + 1) * NF],
                            lhsT=lhs,
                            rhs=wt[1][:, ks * H + hh * HHF + h2 * NF:
                                      ks * H + hh * HHF + (h2 + 1) * NF],
                            start=(ks == 0),
                            stop=(ks == KS2 - 1),
                        )
                dst = ot[3][:, hh * HHF:(hh + 1) * HHF]
                cast_out(dst, ps3[:])
                nc.sync.dma_start(ho[3, :, hh * HHF:(hh + 1) * HHF], dst)

            # Job 4 in three H-pieces (1024, 768, 256): each piece's
            # eviction+DMA hide under the next piece's matmuls, so the tail
            # is only a [P,256] psum: two [P,128] evicts on scalar+vector in
            # parallel and two 64 KB out-issues on the sync+scalar queues in
            # parallel.  (The last 16 matmuls run 1:1 LDW:MM and pay the
            # ~28ns swap tax — ~0.45us — against ~1.3us less tail chain.
            # A first attempt measured +15us but its trace showed 250ns mm
            # deltas = the chip was P0-downclocked to 2.0 GHz that run, not
            # a scheduling fault.)
            QF = NF // 2  # 256
            pieces = [(0, HHF), (HHF, HHF + NF + QF), (HHF + NF + QF, H)]
            for pi, (lo, hi) in enumerate(pieces):
                ps4 = psp.tile([P, HHF], f32, name=f"ps4_{pi}", tag="ps",
                               bufs=4)
                w_pc = hi - lo
                for ks in range(KS2):
                    lhs = xt[4][:, ks * P:(ks + 1) * P]
                    for c0 in range(0, w_pc, NF):
                        c1 = min(c0 + NF, w_pc)
                        nc.tensor.matmul(
                            ps4[:, c0:c1],
                            lhsT=lhs,
                            rhs=wt[1][:, ks * H + lo + c0:ks * H + lo + c1],
                            start=(ks == 0),
                            stop=(ks == KS2 - 1),
                        )
                if pi < 2:
                    dst = ot[4][:, lo:hi]
                    cast_out(dst, ps4[:, :w_pc])
                    nc.sync.dma_start(ho[4, :, lo:hi], dst)
                else:
                    # Single vector evict (the measured scalar->vector split
                    # serialized, +0.7us); output split across two HWDGE
                    # queues so the final 64KB pipelines out in ~0.3us.
                    mid = lo + w_pc // 2
                    nc.vector.tensor_scalar_mul(ot[4][:, lo:hi],
                                                ps4[:, :w_pc], 1.0)
                    nc.sync.dma_start(ho[4, :, lo:mid], ot[4][:, lo:mid])
                    nc.scalar.dma_start(ho[4, :, mid:hi], ot[4][:, mid:hi])
    nc.finalize()
    return nc


def _build_program_fallback(cap: int):
    import concourse.mybir as mybir
    import concourse.tile as tile

    f16 = mybir.dt.float16
    f32 = mybir.dt.float32
    ntok = cap // P

    nc = _new_bacc()
    xT = nc.declare_dram_parameter("xT", [KSUB, P, cap], f16, isOutput=False)
    wk = nc.declare_dram_parameter("wk", [KSUB, P, H], f16, isOutput=False)
    ho = nc.declare_dram_parameter("ho", [ntok, P, H], f16, isOutput=True)

    with tile.TileContext(nc) as tc:
        with tc.tile_pool(name="sb", bufs=1) as sb, \
             tc.tile_pool(name="ps", bufs=2, space="PSUM") as psp:
            xt, wt = [], []
            for k in range(KSUB):
                x_t = sb.tile([P, cap], f16, name=f"x{k}", tag=f"x{k}", bufs=1)
                nc.scalar.dma_start(x_t[:], xT[k, :, :])
                w_t = sb.tile([P, H], f16, name=f"w{k}", tag=f"w{k}", bufs=1)
                nc.sync.dma_start(w_t[:], wk[k, :, :])
                xt.append(x_t)
                wt.append(w_t)

            for t in range(ntok):
                ps = psp.tile([P, H], f32, name=f"ps{t}", tag="ps", bufs=2)
                for k in range(KSUB):
                    lhs = xt[k][:, t * P:(t + 1) * P]
                    for h in range(NH):
                        nc.tensor.matmul(
                            ps[:, h * NF:(h + 1) * NF],
                            lhsT=lhs,
                            rhs=wt[k][:, h * NF:(h + 1) * NF],
                            start=(k == 0),
                            stop=(k == KSUB - 1),
                        )
                o_t = sb.tile([P, H], f16, name=f"o{t}", tag="o", bufs=ntok)
                nc.scalar.copy(o_t[:], ps[:])
                nc.sync.dma_start(ho[t, :, :], o_t[:])
    nc.finalize()
    return nc


def _get_program(key):
    if key not in _prog_cache:
        if key == "jobs":
            _prog_cache[key] = _build_program_jobs()
        else:
            _prog_cache[key] = _build_program_fallback(int(key.split(":")[1]))
    return _prog_cache[key]


def _route(logits, topk):
    """numpy replica of jax.lax.top_k + softmax over selected logits."""
    idx = np.argsort(-logits, axis=-1, kind="stable")[:, :topk]      # [T, topk]
    vals = np.take_along_axis(logits, idx, axis=-1)
    mx = vals.max(-1, keepdims=True)
    gate = np.exp(vals - mx)
    gate = gate / gate.sum(-1, keepdims=True)                        # f32
    return idx, gate


def _pack_groups(tiles_per_expert):
    """Assign (expert, khalf) groups to (core, slot).  Returns
    {(e, kh): (core, slot)} or None if infeasible."""
    groups = []
    for e, ntile in enumerate(tiles_per_expert):
        if ntile == 0:
            continue
        for kh in range(2):
            groups.append((ntile, e, kh))
    groups.sort(reverse=True)
    slots = []  # (capacity, core, slot)
    for c in range(N_CORES):
        slots.append([SLOT_CAP[0], c, 0])
        slots.append([SLOT_CAP[1], c, 1])
    # place largest groups first into the fullest-fitting free slot
    assign = {}
    used = [False] * len(slots)
    for ntile, e, kh in groups:
        best = None
        for i, (cap_s, c, s) in enumerate(slots):
            if used[i] or cap_s < ntile:
                continue
            if best is None or cap_s < slots[best][0]:
                best = i
        if best is None:
            return None
        used[best] = True
        assign[(e, kh)] = (slots[best][1], slots[best][2])
    return assign


def prepare(inputs):
    """Host routing + per-core input construction.

    Returns (nc, launches, combine): launches is a list of per-launch in_maps
    (one dict per core); combine(list_of_per_launch_results) -> final output.
    """
    x = np.asarray(inputs["intermediate_states"])          # [R, TK, I_PR] f16
    w = np.asarray(inputs["w"])                            # [R, E, I_PR, H] f16
    logits = np.asarray(inputs["router_logits"]).astype(np.float32)  # [T, E]
    topk = int(np.asarray(inputs["topk"]))

    T, E_ = logits.shape
    TK = T * topk
    assert x.shape == (R, TK, I_PR) and w.shape == (R, E_, I_PR, H) and E_ == E

    idx, gate = _route(logits, topk)
    flat_e = idx.reshape(-1)                               # expert of tk
    counts = np.bincount(flat_e, minlength=E)
    starts = np.zeros(E + 1, np.int64)
    starts[1:] = np.cumsum(counts)
    order = np.argsort(flat_e, kind="stable")              # tks sorted by expert
    g_flat = gate.reshape(TK).astype(np.float32)
    xf = np.ascontiguousarray(x.transpose(1, 0, 2)).reshape(TK, K)  # [TK, 4096]

    tiles_per_expert = [-(-int(c) // P) for c in counts]
    assign = _pack_groups(tiles_per_expert)
    if assign is not None:
        return _prepare_jobs(w, xf, g_flat, order, starts, counts,
                             tiles_per_expert, assign, topk, T)
    return _prepare_fallback(w, xf, g_flat, order, starts, counts, topk, T)


def _prepare_jobs(w, xf, g_flat, order, starts, counts, tiles_per_expert,
                  assign, topk, T):
    TK = T * topk
    nc = _get_program("jobs")

    xj = np.zeros((N_CORES, JOBS, P, KS2, P), np.float16)
    whs = np.zeros((N_CORES, 2, KS2, P, H), np.float16)
    # pos[kh][tk] = row index in the assembled h for token tk's kh-half partial
    pos = np.zeros((2, TK), np.int64)

    job_base = {0: 0, 1: SLOT_CAP[0]}
    for (e, kh), (core, slot) in assign.items():
        toks_e = order[starts[e]:starts[e + 1]]            # ascending tks
        # w slice: K-half kh of expert e -> [2048, 2048]
        wsl = np.ascontiguousarray(w[2 * kh:2 * kh + 2, e].reshape(KH, H))
        whs[core, slot] = wsl.reshape(KS2, P, H)
        for tt in range(tiles_per_expert[e]):
            j = job_base[slot] + tt
            toks = toks_e[tt * P:(tt + 1) * P]
            n = len(toks)
            # stationary layout [P(krow), KS2, P(tok)] from gated x K-half kh
            xs = (xf[toks, kh * KH:(kh + 1) * KH].astype(np.float32)
                  * g_flat[toks, None]).astype(np.float16)          # [n, 2048]
            xj[core, j, :, :, :n] = xs.reshape(n, KS2, P).transpose(2, 1, 0)
            pos[kh, toks] = (core * JOBS + j) * P + np.arange(n)

    launches = [[{"xj": xj[c].reshape(JOBS, P, KS2 * P), "wh": whs[c]}
                 for c in range(N_CORES)]]

    def combine(all_results):
        res = all_results[0]
        h_all = np.concatenate(
            [res[c]["ho"].reshape(JOBS * P, H) for c in range(N_CORES)], axis=0)
        y = np.zeros((T, H), np.float32)
        for kh in range(2):
            for kk in range(topk):
                y += h_all[pos[kh, kk::topk]].astype(np.float32)
        return y.astype(np.float16).reshape(R, T // R, H)

    return nc, launches, combine


def _prepare_fallback(w, xf, g_flat, order, starts, counts, topk, T):
    TK = T * topk
    cap_needed = -(-max(int(counts.max()), 1) // P) * P
    cap_launch = min(cap_needed, CAP_FB)
    n_launch = -(-cap_needed // cap_launch)
    cap_total = n_launch * cap_launch
    ntok_l = cap_launch // P

    nc = _get_program(f"fb:{cap_launch}")

    pos = np.empty(TK, np.int64)
    for e in range(E):
        toks = order[starts[e]:starts[e + 1]]
        pos[toks] = e * cap_total + np.arange(len(toks))

    launches = []
    for j in range(n_launch):
        in_maps = []
        for e in range(E):
            toks = order[starts[e]:starts[e + 1]][j * cap_launch:(j + 1) * cap_launch]
            c = len(toks)
            xTe = np.zeros((K, cap_launch), np.float16)
            if c:
                xg = (xf[toks].astype(np.float32)
                      * g_flat[toks, None]).astype(np.float16)
                xTe[:, :c] = xg.T
            in_maps.append({
                "xT": np.ascontiguousarray(xTe.reshape(KSUB, P, cap_launch)),
                "wk": np.ascontiguousarray(w[:, e].reshape(K, H)).reshape(KSUB, P, H),
            })
        launches.append(in_maps)

    def combine(all_results):
        h_all = np.empty((E * cap_total, H), np.float16)
        for j, res in enumerate(all_results):
            for e in range(E):
                h_all[e * cap_total + j * cap_launch:
                      e * cap_total + (j + 1) * cap_launch] = \
                    res[e]["ho"].reshape(cap_launch, H)
        y = h_all[pos[0::topk]].astype(np.float32)
        for kk in range(1, topk):
            y += h_all[pos[kk::topk]].astype(np.float32)
        return y.astype(np.float16).reshape(R, T // R, H)

    return nc, launches, combine


def kernel(**inputs) -> np.ndarray:
    nc, launches, combine = prepare(inputs)
    from concourse.bass_utils import run_bass_kernel_spmd

    all_results = []
    for in_maps in launches:
        res = run_bass_kernel_spmd(nc, in_maps, core_ids=list(range(N_CORES)))
        all_results.append(res.results)
    return combine(all_results)

